# revision 1
# baseline (speedup 1.0000x reference)
"""GNN linear-attention kernel for Trainium2 (8 NeuronCores).

Sharding: data-parallel over batch B=8 -- one graph (N=2048 nodes) per
NeuronCore; parameters replicated. Inputs are full (unsharded) numpy
arrays; output is the full (B, N, O) float32 array.
"""

import numpy as np

B, N, D, O = 8, 2048, 128, 128

_compiled = {}


def _get_fn():
    import jax
    import jax.numpy as jnp

    if "fn" in _compiled:
        return _compiled["fn"]

    def f(x_b, A_b, W_qk, b_qk, W_l, b_l, W_r, W_d, b_d):
        d = x_b.shape[-1]
        deg = jnp.sum(A_b, axis=-1, keepdims=True)
        gate = jax.nn.sigmoid(deg @ W_d + b_d)
        xg = x_b * gate
        QK = jax.nn.sigmoid(xg @ W_qk + b_qk)
        scores = (QK @ QK.T) / jnp.sqrt(jnp.float32(d))
        scores = scores * A_b
        attn = scores / (jnp.sum(scores, axis=-1, keepdims=True) + 1e-6)
        agg = attn @ xg
        out = agg @ W_l + b_l + xg @ W_r
        nrm = jnp.linalg.norm(out, axis=-1, keepdims=True)
        return out / jnp.maximum(nrm, 1e-12)

    _compiled["fn"] = jax.jit(f)
    return _compiled["fn"]


def kernel(x, A, W_qk, b_qk, W_l, b_l, W_r, W_d, b_d):
    import jax

    fn = _get_fn()
    devs = jax.devices()
    ndev = min(len(devs), x.shape[0])
    weights = (W_qk, b_qk, W_l, b_l, W_r, W_d, b_d)

    # Dispatch one graph per NeuronCore (async), then gather.
    futs = []
    for b in range(x.shape[0]):
        dev = devs[b % ndev]
        args = [jax.device_put(np.asarray(t), dev) for t in (x[b], A[b]) + weights]
        futs.append(fn(*args))
    out = np.stack([np.asarray(f_) for f_ in futs], axis=0)
    return out.astype(np.float32)


# revision 2
# speedup vs baseline: 1.2007x; 1.2007x over previous
"""GNN linear-attention kernel for Trainium2 (8 NeuronCores).

Sharding: data-parallel over batch B=8 -- one graph (N=2048 nodes) per
NeuronCore; parameters replicated. Inputs are full (unsharded) numpy
arrays; output is the full (B, N, O) float32 array.
"""

import numpy as np

B, N, D, O = 8, 2048, 128, 128

_compiled = {}


def _get_fn():
    import jax
    import jax.numpy as jnp

    if "fn" in _compiled:
        return _compiled["fn"]

    def f(x_b, A_b, W_qk, b_qk, W_l, b_l, W_r, W_d, b_d):
        d = x_b.shape[-1]
        deg = jnp.sum(A_b, axis=-1, keepdims=True)
        gate = jax.nn.sigmoid(deg @ W_d + b_d)
        xg = x_b * gate
        QK = jax.nn.sigmoid(xg @ W_qk + b_qk)
        scores = (QK @ QK.T) / jnp.sqrt(jnp.float32(d))
        scores = scores * A_b
        attn = scores / (jnp.sum(scores, axis=-1, keepdims=True) + 1e-6)
        agg = attn @ xg
        out = agg @ W_l + b_l + xg @ W_r
        nrm = jnp.linalg.norm(out, axis=-1, keepdims=True)
        return out / jnp.maximum(nrm, 1e-12)

    _compiled["fn"] = jax.jit(f)
    _compiled["pfn"] = jax.pmap(
        f, in_axes=(0, 0, None, None, None, None, None, None, None)
    )
    return _compiled["fn"]


def _run_loop(x, A, weights):
    import jax

    fn = _get_fn()
    devs = jax.devices()
    ndev = min(len(devs), x.shape[0])
    futs = []
    for b in range(x.shape[0]):
        dev = devs[b % ndev]
        args = [jax.device_put(np.asarray(t), dev) for t in (x[b], A[b]) + weights]
        futs.append(fn(*args))
    return np.stack([np.asarray(f_) for f_ in futs], axis=0)


def kernel(x, A, W_qk, b_qk, W_l, b_l, W_r, W_d, b_d):
    weights = (W_qk, b_qk, W_l, b_l, W_r, W_d, b_d)
    _get_fn()
    try:
        # One parallel dispatch: one graph per NeuronCore.
        out = np.asarray(_compiled["pfn"](x, A, *weights))
    except Exception:
        out = _run_loop(x, A, weights)
    return out.astype(np.float32)
